# revision 11
# baseline (speedup 1.0000x reference)
"""Trainium2 Bass kernel for CondGruAttentionSeq (T=64, B=32, V=50257).

Strategy (8 NeuronCores):
  - Recurrent part (cond-GRU encoder, Bahdanau attention + main GRU) is
    data-parallel over batch: each core owns 4 of the 32 batch elements.
  - The dominant dense head ([T*B, 896] @ [896, V] + log_softmax) is
    tensor-parallel over the vocab: the per-step feature rows
    z_t = [c_t, x_emb_t, h_{t-1}] are all-gathered (bf16) across cores,
    each core computes logits for its ~V/8 vocab shard, and the softmax
    normalizer is all-reduced (sum of exp; scores/logits are tiny so no
    max subtraction is needed for stability).
Host side does the embedding gathers / sharding / transposes and the final
unshard; all model compute runs on device.
"""
import numpy as np
import ml_dtypes
from contextlib import ExitStack

import concourse.bass as bass
import concourse.tile as tile
from concourse import bacc, mybir
from concourse import bass_utils

F32 = mybir.dt.float32
BF16 = mybir.dt.bfloat16
AX = mybir.AxisListType
ALU = mybir.AluOpType
ACTF = mybir.ActivationFunctionType

# Problem constants (hardcoded per contract).
V, H, EMB, E = 50257, 512, 256, 128
T, B, TC, S = 64, 32, 32, 35
NC_ = 8                 # cores
BL = B // NC_           # batch per core = 4
R = T * BL              # local feature rows = 256
RG = T * B              # global feature rows = 2048
KD = E + EMB + H        # dense head contraction = 896
KT = KD // 128          # = 7 k-tiles
VSH = (V + NC_ - 1) // NC_  # padded vocab shard = 6283
NT_FULL, NT_LAST = VSH // 512, VSH % 512    # 12 tiles of 512 + 139
NNT = NT_FULL + (1 if NT_LAST else 0)       # 13 n-tiles per shard
NCHUNK = RG // 128      # 16 row chunks in the big GEMM
BIAS_PAD = -30.0        # pad vocab bias => exp() contributes ~0


def _bf16(x):
    return np.ascontiguousarray(np.asarray(x).astype(ml_dtypes.bfloat16))


def _f32(x):
    return np.ascontiguousarray(np.asarray(x).astype(np.float32))


def _tiles_T(w, kparts, jparts):
    """w: [M, K] -> stationary tile array [128, kparts, jparts, 128] with
    arr[p, kt, jt, m] = w[jt*128+m, kt*128+p] (i.e. w.T tiled)."""
    Kd, Md = kparts * 128, jparts * 128
    assert w.shape == (Md, Kd), (w.shape, (Md, Kd))
    wt = np.ascontiguousarray(w.T)                  # [K, M]
    wt = wt.reshape(kparts, 128, jparts, 128)       # [kt, p, jt, m]
    return np.ascontiguousarray(wt.transpose(1, 0, 2, 3))


def _btile(b, jparts):
    """bias [Md] -> [128, jparts] with arr[p, jt] = b[jt*128+p]"""
    return np.ascontiguousarray(np.asarray(b, np.float32)
                                .reshape(jparts, 128).T)


def build_program():
    nc = bacc.Bacc("TRN2", target_bir_lowering=False, debug=False,
                   num_devices=NC_)

    inp = {}

    def din(name, shape, dtype):
        inp[name] = nc.dram_tensor(name, list(shape), dtype,
                                   kind="ExternalInput")
        return inp[name]

    # --- inputs (per-core shards; same shapes on all cores) ---
    din("wcomb", [128, 4 * 16 * 128], BF16)   # [Whh;W_h]^T stationary tiles
    din("wih", [128, 2 * 12 * 128], BF16)     # gru_Wih^T tiles
    din("cw", [128, 6 * 128], BF16)           # cgru Wih^T (3) + Whh^T (3)
    din("wet", [128, 4 * 128], BF16)          # W_e^T tiles
    din("vt", [128, 4], BF16)                 # v tiles
    din("ident", [128, 128], F32)             # identity for PE transpose
    din("xet", [128, 2 * R], BF16)            # x_emb^T (local rows)
    din("cseqt", [128, TC * BL], BF16)        # cseq_in^T
    din("condt", [128, BL * 3], F32)          # cond emb ^T, cols (b, s)
    din("xwb", [128, 12], F32)                # gru bih + bhh(rz part) tiles
    din("gbhhn", [128, 4], F32)               # gru bhh n-part tiles
    din("cxwb", [128, 3], F32)                # cgru bih + bhh(rz) tiles
    din("cbhhn", [128, 1], F32)               # cgru bhh n-part
    din("ab", [128, 4], F32)                  # attn_b tiles
    din("wd", [128, KT * VSH], BF16)          # dense_W^T vocab shard
    din("db", [VSH], BF16)                    # dense_b shard (padded -30)

    out_pred = nc.dram_tensor("out_pred", [T, B, VSH], F32,
                              kind="ExternalOutput")
    out_h = nc.dram_tensor("out_h", [128, 16], F32, kind="ExternalOutput")
    out_w = nc.dram_tensor("out_w", [T * BL * S], F32, kind="ExternalOutput")

    # --- internal DRAM for collectives ---
    zt_in = nc.dram_tensor("zt_in", [KD, R], BF16, kind="Internal")
    zt_out = nc.dram_tensor("zt_out", [NC_ * KD, R], BF16, kind="Internal",
                            addr_space="Shared")
    ss_in = nc.dram_tensor("ss_in", [NCHUNK, 128], F32, kind="Internal")
    ss_out = nc.dram_tensor("ss_out", [NCHUNK, 128], F32, kind="Internal",
                            addr_space="Shared")
    groups = [list(range(NC_))]

    with tile.TileContext(nc) as tc, ExitStack() as ctx:
        const = ctx.enter_context(tc.tile_pool(name="const", bufs=1))
        dma = nc.sync.dma_start

        # ---- tensors that live through the whole kernel ----
        wd_sb = const.tile([128, KT, VSH], BF16)
        dma(wd_sb[:], inp["wd"].ap().rearrange("p (k v) -> p k v", k=KT))
        brep = const.tile([128, VSH], BF16)
        dma(brep[:], inp["db"].ap().unsqueeze(0).broadcast_to([128, VSH]))

        # Z^T local slice [128, kt, r]; k-tiles: 0=c, 1-2=xe, 3-6=h_prev
        zts = const.tile([128, KT, R], BF16)
        dma(zts[:, 1:3, :], inp["xet"].ap().rearrange("p (k r) -> p k r",
                                                      k=2))
        nc.vector.memset(zts[:, 3:7, 0:BL], 0.0)    # h_{-1} = 0 for row t=0

        enc_s = const.tile([128, BL * 128], F32)    # enc in [s,(b,e)] rows<35

        # ================= recurrent phase =================
        with ExitStack() as rctx:
            rec = rctx.enter_context(tc.tile_pool(name="rec", bufs=1))

            wcomb = rec.tile([128, 64, 128], BF16)
            dma(wcomb[:], inp["wcomb"].ap().rearrange("p (t m) -> p t m",
                                                      m=128))
            wih = rec.tile([128, 24, 128], BF16)
            dma(wih[:], inp["wih"].ap().rearrange("p (t m) -> p t m", m=128))
            cw = rec.tile([128, 6, 128], BF16)
            dma(cw[:], inp["cw"].ap().rearrange("p (t m) -> p t m", m=128))
            wet = rec.tile([128, 4, 128], BF16)
            dma(wet[:], inp["wet"].ap().rearrange("p (t m) -> p t m", m=128))
            vt = rec.tile([128, 4], BF16)
            dma(vt[:], inp["vt"].ap())
            ident = rec.tile([128, 128], F32)
            dma(ident[:], inp["ident"].ap())
            cseqt = rec.tile([128, TC * BL], BF16)
            dma(cseqt[:], inp["cseqt"].ap())
            xwb = rec.tile([128, 12], F32)
            dma(xwb[:], inp["xwb"].ap())
            gbhhn = rec.tile([128, 4], F32)
            dma(gbhhn[:], inp["gbhhn"].ap())
            cxwb = rec.tile([128, 3], F32)
            dma(cxwb[:], inp["cxwb"].ap())
            cbhhn = rec.tile([128, 1], F32)
            dma(cbhhn[:], inp["cbhhn"].ap())
            ab = rec.tile([128, 4], F32)
            dma(ab[:], inp["ab"].ap())

            encT = rec.tile([128, BL * S], F32)       # cols b*35+s
            dma(encT[:].rearrange("p (b s) -> p b s", s=S)[:, :, 0:3],
                inp["condt"].ap().rearrange("p (b s) -> p b s", s=3))

            xw_sb = rec.tile([128, 12, R], F32)       # xw + bih + bhh_rz
            ep_sb = rec.tile([128, 4, BL * S], F32)   # enc_proj + attn_b

            # ---- xw precompute ----
            with tc.tile_pool(name="ppsum", bufs=2, space="PSUM") as pp:
                for jt in range(12):
                    ps = pp.tile([128, R], F32, tag="pp")
                    for kt in range(2):
                        nc.tensor.matmul(ps[:], wih[:, kt * 12 + jt, :],
                                         zts[:, 1 + kt, :],
                                         start=(kt == 0), stop=(kt == 1))
                    nc.vector.tensor_scalar(xw_sb[:, jt, :], ps[:],
                                            xwb[:, jt:jt + 1], None, ALU.add)

            # ---- cond-seq GRU (32 steps) ----
            with tc.tile_pool(name="cg", bufs=2) as cgp, \
                 tc.tile_pool(name="cgps", bufs=2, space="PSUM") as cgps:
                cxw_sb = rec.tile([128, 3, TC * BL], F32)
                for g in range(3):
                    ps = cgps.tile([128, TC * BL], F32, tag="cxw")
                    nc.tensor.matmul(ps[:], cw[:, g, :], cseqt[:],
                                     start=True, stop=True)
                    nc.vector.tensor_scalar(cxw_sb[:, g, :], ps[:],
                                            cxwb[:, g:g + 1], None, ALU.add)
                ch = cgp.tile([128, BL], F32, tag="ch")
                nc.vector.memset(ch[:], 0.0)
                chb = cgp.tile([128, BL], BF16, tag="chb")
                nc.vector.memset(chb[:], 0.0)
                for t in range(TC):
                    pg = cgps.tile([128, 3 * BL], F32, tag="cpg")
                    for g in range(3):
                        nc.tensor.matmul(pg[:, g * BL:(g + 1) * BL],
                                         cw[:, 3 + g, :], chb[:],
                                         start=True, stop=True)
                    arz = cgp.tile([128, 2 * BL], F32, tag="carz")
                    nc.vector.tensor_tensor(
                        arz[:].rearrange("p (g b) -> p g b", g=2),
                        pg[:, 0:2 * BL].rearrange("p (g b) -> p g b", g=2),
                        cxw_sb[:, 0:2, t * BL:(t + 1) * BL], ALU.add)
                    nc.scalar.activation(arz[:], arz[:], ACTF.Sigmoid)
                    hwn = cgp.tile([128, BL], F32, tag="chwn")
                    nc.vector.tensor_scalar(hwn[:], pg[:, 2 * BL:3 * BL],
                                            cbhhn[:, 0:1], None, ALU.add)
                    na = cgp.tile([128, BL], F32, tag="cna")
                    nc.vector.tensor_tensor(na[:], arz[:, 0:BL], hwn[:],
                                            ALU.mult)
                    nc.vector.tensor_tensor(na[:], na[:],
                                            cxw_sb[:, 2,
                                                   t * BL:(t + 1) * BL],
                                            ALU.add)
                    nc.scalar.activation(na[:], na[:], ACTF.Tanh)
                    d = cgp.tile([128, BL], F32, tag="cd")
                    nc.vector.tensor_tensor(d[:], ch[:], na[:], ALU.subtract)
                    nc.vector.tensor_tensor(d[:], arz[:, BL:2 * BL], d[:],
                                            ALU.mult)
                    ch = cgp.tile([128, BL], F32, tag="ch")
                    nc.vector.tensor_tensor(ch[:], d[:], na[:], ALU.add)
                    chb = cgp.tile([128, BL], BF16, tag="chb")
                    nc.vector.tensor_copy(chb[:], ch[:])
                    nc.vector.tensor_copy(
                        encT[:].rearrange("p (b s) -> p b s", s=S)
                        [:, :, 3 + t],
                        ch[:])

            # ---- enc_proj & enc_s (one-time) ----
            with tc.tile_pool(name="ept", bufs=2) as ept, \
                 tc.tile_pool(name="eps", bufs=2, space="PSUM") as epp:
                encTb = ept.tile([128, BL * S], BF16, tag="encTb")
                nc.vector.tensor_copy(encTb[:], encT[:])
                for jt in range(4):
                    ps = epp.tile([128, BL * S], F32, tag="ep")
                    nc.tensor.matmul(ps[:], wet[:, jt, :], encTb[:],
                                     start=True, stop=True)
                    nc.vector.tensor_scalar(ep_sb[:, jt, :], ps[:],
                                            ab[:, jt:jt + 1], None, ALU.add)
                for b in range(BL):
                    pt = epp.tile([S, 128], F32, tag="et")
                    nc.tensor.transpose(pt[:], encT[:, b * S:(b + 1) * S],
                                        ident[:])
                    nc.vector.tensor_copy(
                        enc_s[0:S, b * 128:(b + 1) * 128], pt[:])

            # ---- main recurrence (64 steps) ----
            with tc.tile_pool(name="mg", bufs=3) as mgp, \
                 tc.tile_pool(name="mgps", bufs=2, space="PSUM") as mgps, \
                 tc.tile_pool(name="msps", bufs=2, space="PSUM") as msps:
                hT = mgp.tile([128, 16], F32, tag="hT")
                nc.vector.memset(hT[:], 0.0)
                hTb = mgp.tile([128, 16], BF16, tag="hTb")
                nc.vector.memset(hTb[:], 0.0)
                for t in range(T):
                    pga = mgps.tile([128, 64], F32, tag="pga")
                    for jt in range(16):
                        for kt in range(4):
                            nc.tensor.matmul(
                                pga[:, jt * 4:(jt + 1) * 4],
                                wcomb[:, kt * 16 + jt, :],
                                hTb[:, kt * 4:(kt + 1) * 4],
                                start=(kt == 0), stop=(kt == 3))
                    # --- GRU gates (cols 0:48) ---
                    hwn = mgp.tile([128, 16], F32, tag="hwn")
                    nc.vector.tensor_tensor(
                        hwn[:].rearrange("p (k b) -> p k b", k=4),
                        pga[:, 32:48].rearrange("p (k b) -> p k b", k=4),
                        gbhhn[:].unsqueeze(2).broadcast_to([128, 4, BL]),
                        ALU.add)
                    arz = mgp.tile([128, 32], F32, tag="arz")
                    nc.vector.tensor_tensor(
                        arz[:].rearrange("p (k b) -> p k b", k=8),
                        pga[:, 0:32].rearrange("p (k b) -> p k b", k=8),
                        xw_sb[:, 0:8, t * BL:(t + 1) * BL], ALU.add)
                    nc.scalar.activation(arz[:], arz[:], ACTF.Sigmoid)
                    na = mgp.tile([128, 16], F32, tag="na")
                    nc.vector.tensor_tensor(na[:], arz[:, 0:16], hwn[:],
                                            ALU.mult)
                    nc.vector.tensor_tensor(
                        na[:].rearrange("p (k b) -> p k b", k=4),
                        na[:].rearrange("p (k b) -> p k b", k=4),
                        xw_sb[:, 8:12, t * BL:(t + 1) * BL], ALU.add)
                    nc.scalar.activation(na[:], na[:], ACTF.Tanh)
                    d = mgp.tile([128, 16], F32, tag="d")
                    nc.vector.tensor_tensor(d[:], hT[:], na[:], ALU.subtract)
                    nc.vector.tensor_tensor(d[:], arz[:, 16:32], d[:],
                                            ALU.mult)
                    hT_new = mgp.tile([128, 16], F32, tag="hT")
                    nc.vector.tensor_tensor(hT_new[:], d[:], na[:], ALU.add)
                    hTb = mgp.tile([128, 16], BF16, tag="hTb")
                    nc.vector.tensor_copy(hTb[:], hT_new[:])
                    if t + 1 < T:
                        nc.scalar.copy(
                            zts[:, 3:7, (t + 1) * BL:(t + 2) * BL],
                            hT_new[:].rearrange("p (k b) -> p k b", k=4))
                    # --- attention (cols 48:64 = h @ W_h^T + ...) ---
                    ta = mgp.tile([128, 4, BL * S], BF16, tag="ta")
                    nc.vector.tensor_tensor(
                        ta[:].rearrange("p k (b s) -> p k b s", s=S),
                        ep_sb[:].rearrange("p k (b s) -> p k b s", s=S),
                        pga[:, 48:64].rearrange("p (k b) -> p k b", k=4)
                        .unsqueeze(3).broadcast_to([128, 4, BL, S]), ALU.add)
                    nc.scalar.activation(ta[:], ta[:], ACTF.Tanh)
                    pss = msps.tile([1, BL * S], F32, tag="pss")
                    for kt in range(4):
                        nc.tensor.matmul(pss[:], vt[:, kt:kt + 1],
                                         ta[:, kt, :],
                                         start=(kt == 0), stop=(kt == 3))
                    es = mgp.tile([1, BL * S], F32, tag="es")
                    nc.scalar.activation(es[:], pss[:], ACTF.Exp)
                    ssum = mgp.tile([1, BL], F32, tag="ssum")
                    nc.vector.tensor_reduce(
                        ssum[:], es[:].rearrange("p (b s) -> p b s", s=S),
                        AX.X, ALU.add)
                    rs = mgp.tile([1, BL], F32, tag="rs")
                    nc.vector.reciprocal(rs[:], ssum[:])
                    wrow = mgp.tile([1, BL * S], F32, tag="wrow")
                    nc.vector.tensor_tensor(
                        wrow[:].rearrange("p (b s) -> p b s", s=S),
                        es[:].rearrange("p (b s) -> p b s", s=S),
                        rs[:].unsqueeze(2).broadcast_to([1, BL, S]),
                        ALU.mult)
                    dma(out_w.ap()[t * BL * S:(t + 1) * BL * S], wrow[:])
                    hT = hT_new
                dma(out_h.ap(), hT[:])

            # ---- batched context vectors ----
            with tc.tile_pool(name="cpost", bufs=1) as cp, \
                 tc.tile_pool(name="cps", bufs=2, space="PSUM") as cps:
                wT = cp.tile([S, BL * T], F32)   # [s, (b, t)]
                for b in range(BL):
                    dma(wT[:, b * T:(b + 1) * T],
                        out_w.ap().rearrange("(t b s) -> s b t",
                                             s=S, b=BL)[:, b, :])
                for b in range(BL):
                    ps = cps.tile([128, T], F32, tag="cc")
                    nc.tensor.matmul(ps[:],
                                     enc_s[0:S, b * 128:(b + 1) * 128],
                                     wT[:, b * T:(b + 1) * T],
                                     start=True, stop=True)
                    nc.scalar.copy(
                        zts[:, 0, :].rearrange("p (t b) -> p t b", b=BL)
                        [:, :, b],
                        ps[:])

        # ================= allgather Z =================
        dma(zt_in.ap().rearrange("(k p) r -> p k r", p=128), zts[:])
        nc.gpsimd.collective_compute(
            "AllGather", ALU.bypass, replica_groups=groups,
            ins=[zt_in.ap()], outs=[zt_out.ap()])

        zgp = ctx.enter_context(tc.tile_pool(name="zgp", bufs=1))
        ztg = zgp.tile([128, KT, NC_ * R], BF16)     # cols (kt, c, r)
        for kt in range(KT):
            dma(ztg[:, kt, :],
                zt_out.ap().rearrange("(c k p) r -> p k c r",
                                      p=128, k=KT)[:, kt, :, :])

        # ================= big GEMM + log_softmax =================
        nsz = [512] * NT_FULL + ([NT_LAST] if NT_LAST else [])
        noff = np.cumsum([0] + nsz).tolist()
        with tc.tile_pool(name="gps", bufs=8, space="PSUM") as gps, \
             tc.tile_pool(name="glb", bufs=3) as glb, \
             tc.tile_pool(name="gsm", bufs=4) as gsm, \
             tc.tile_pool(name="gout", bufs=4) as gout:
            lbs, stats = {}, {}
            for m in range(NCHUNK):
                lb = glb.tile([128, VSH], BF16, tag="lb")
                lbs[m] = lb
                parts = gsm.tile([128, NNT], F32, tag="parts")
                for wave in (list(range(8)), list(range(8, NNT))):
                    pts = {n: gps.tile([128, nsz[n]], F32, tag="g",
                                       name=f"g_{m}_{n}")
                           for n in wave}
                    for kt in range(KT):
                        for n in wave:
                            nc.tensor.matmul(
                                pts[n][:],
                                ztg[:, kt, m * 128:(m + 1) * 128],
                                wd_sb[:, kt, noff[n]:noff[n + 1]],
                                start=(kt == 0), stop=(kt == KT - 1))
                    for n in wave:
                        sl = slice(noff[n], noff[n + 1])
                        nc.vector.tensor_tensor(lb[:, sl], pts[n][:],
                                                brep[:, sl], ALU.add)
                        scr = gsm.tile([128, 512], F32, tag="scr")
                        nc.scalar.activation(scr[:, 0:nsz[n]], lb[:, sl],
                                             ACTF.Exp,
                                             accum_out=parts[:, n:n + 1])
                tot = gsm.tile([128, 1], F32, tag="tot")
                nc.vector.tensor_reduce(tot[:], parts[:], AX.X, ALU.add)
                stats[m] = tot
                if m % 2 == 1:
                    g = m // 2
                    st2 = gsm.tile([128, 2], F32, tag="st2")
                    nc.vector.tensor_copy(st2[:, 0:1], stats[m - 1][:])
                    nc.vector.tensor_copy(st2[:, 1:2], stats[m][:])
                    dma(ss_in.ap()[2 * g:2 * g + 2, :]
                        .rearrange("g p -> p g"), st2[:])
                    nc.gpsimd.collective_compute(
                        "AllReduce", ALU.add, replica_groups=groups,
                        ins=[ss_in.ap()[2 * g:2 * g + 2, :]],
                        outs=[ss_out.ap()[2 * g:2 * g + 2, :]])
                    gs = gsm.tile([128, 2], F32, tag="gs")
                    dma(gs[:], ss_out.ap()[2 * g:2 * g + 2, :]
                        .rearrange("g p -> p g"))
                    logz = gsm.tile([128, 2], F32, tag="logz")
                    nc.scalar.activation(logz[:], gs[:], ACTF.Ln)
                    for mi, mc in enumerate((m - 1, m)):
                        csrc, rh = mc // 2, mc % 2
                        t0 = rh * 32
                        dst = out_pred.ap()[t0:t0 + 32,
                                            csrc * BL:(csrc + 1) * BL, :]
                        for n in range(NNT):
                            sl = slice(noff[n], noff[n + 1])
                            ot = gout.tile([128, 512], F32, tag="ot")
                            nc.vector.tensor_scalar(
                                ot[:, 0:nsz[n]], lbs[mc][:, sl],
                                logz[:, mi:mi + 1], None, ALU.subtract)
                            dma(dst[:, :, sl], ot[:, 0:nsz[n]])
                    del lbs[m - 1], lbs[m], stats[m - 1], stats[m]

    nc.compile()
    return nc


_CACHE = {}


def prep_inputs(inputs):
    """Full numpy inputs -> list of 8 per-core in_maps."""
    g = {k: np.asarray(v) for k, v in inputs.items()}
    embedding = _f32(g["embedding"])
    x_emb = embedding[np.asarray(g["input_data"], np.int64)]   # [T,B,EMB]
    tables = (g["emb_cond_0"], g["emb_cond_1"], g["emb_cond_2"])
    cond = np.asarray(g["condition"], np.int64)
    cond_emb = np.stack([_f32(tables[i])[cond[i]] for i in range(3)], 0)
    cseq_in = _f32(g["cond_seq_emb"])[np.asarray(g["cond_seq"], np.int64)]

    Wcat = np.concatenate([_f32(g["gru_Whh"]), _f32(g["attn_W"])[:, E:]], 0)
    wcomb = _bf16(_tiles_T(Wcat, 4, 16).reshape(128, -1))
    wih = _bf16(_tiles_T(_f32(g["gru_Wih"]), 2, 12).reshape(128, -1))
    cwa = np.concatenate([
        _tiles_T(_f32(g["cgru_Wih"]), 1, 3),
        _tiles_T(_f32(g["cgru_Whh"]), 1, 3)], axis=2)
    cw = _bf16(cwa.reshape(128, -1))
    wet = _bf16(_tiles_T(np.ascontiguousarray(_f32(g["attn_W"])[:, :E]),
                         1, 4).reshape(128, -1))
    vtile = _btile(_f32(g["v_W"])[0], 4)
    ident = np.eye(128, dtype=np.float32)

    gbih, gbhh = _f32(g["gru_bih"]), _f32(g["gru_bhh"])
    xwb = _btile(gbih, 12) + np.concatenate(
        [_btile(gbhh, 12)[:, :8], np.zeros((128, 4), np.float32)], 1)
    gbhhn = np.ascontiguousarray(_btile(gbhh, 12)[:, 8:12])
    cbih, cbhh = _f32(g["cgru_bih"]), _f32(g["cgru_bhh"])
    cxwb = _btile(cbih, 3) + np.concatenate(
        [_btile(cbhh, 3)[:, :2], np.zeros((128, 1), np.float32)], 1)
    cbhhn = np.ascontiguousarray(_btile(cbhh, 3)[:, 2:3])
    ab = _btile(_f32(g["attn_b"]), 4)

    dW = _f32(g["dense_W"])
    dB = _f32(g["dense_b"])
    dWp = np.zeros((NC_ * VSH, KD), np.float32)
    dBp = np.full((NC_ * VSH,), BIAS_PAD, np.float32)
    dWp[:V] = dW
    dBp[:V] = dB

    in_maps = []
    for c in range(NC_):
        b0 = c * BL
        xe = x_emb[:, b0:b0 + BL, :].reshape(R, EMB)          # rows (t,b)
        # xet[p, kt, r] = xe[r, kt*128+p]
        xet = _bf16(np.ascontiguousarray(xe.T).reshape(2, 128, R)
                    .transpose(1, 0, 2).reshape(128, 2 * R))
        csl = cseq_in[:, b0:b0 + BL, :]                       # [TC,BL,E]
        cseqt = _bf16(np.ascontiguousarray(csl.reshape(TC * BL, E).T))
        condt = _f32(cond_emb[:, b0:b0 + BL, :].transpose(2, 1, 0)
                     .reshape(128, BL * 3))
        wdp = dWp[c * VSH:(c + 1) * VSH]                      # [VSH, KD]
        wd = _bf16(np.ascontiguousarray(wdp.T).reshape(KT, 128, VSH)
                   .transpose(1, 0, 2).reshape(128, KT * VSH))
        db = _bf16(dBp[c * VSH:(c + 1) * VSH])
        in_maps.append({
            "wcomb": wcomb, "wih": wih, "cw": cw, "wet": wet,
            "vt": _bf16(vtile), "ident": ident, "xet": xet,
            "cseqt": cseqt, "condt": condt, "xwb": _f32(xwb),
            "gbhhn": gbhhn, "cxwb": _f32(cxwb), "cbhhn": cbhhn,
            "ab": ab, "wd": wd, "db": db,
        })
    return in_maps


def unshard(results):
    """per-core result dicts -> (total_pred, hT, weights) full outputs."""
    preds = []
    for c in range(NC_):
        p = results[c]["out_pred"]                  # [T, B, VSH]
        lo = c * VSH
        hi = min(V, lo + VSH)
        preds.append(p[:, :, :hi - lo])
    total_pred = np.concatenate(preds, axis=2)      # [T, B, V]
    hs, ws = [], []
    for c in range(NC_):
        hm = results[c]["out_h"].reshape(128, 4, BL)      # [e, k, b]
        hs.append(hm.transpose(2, 1, 0).reshape(BL, H))   # [b, (k,e)]
        ws.append(results[c]["out_w"].reshape(T, BL, S))
    hT = np.concatenate(hs, 0)[None]                # [1, B, H]
    weights = np.concatenate(ws, 1)                 # [T, B, S]
    return total_pred, hT, weights


def kernel(**inputs):
    if "nc" not in _CACHE:
        _CACHE["nc"] = build_program()
    nc = _CACHE["nc"]
    in_maps = prep_inputs(inputs)
    res = bass_utils.run_bass_kernel_spmd(nc, in_maps,
                                          core_ids=list(range(NC_)))
    return unshard(res.results)
